# revision 1
# baseline (speedup 1.0000x reference)
"""Chamfer distance (L2, squared) Bass kernel for Trainium2.

Problem: xyz1 (4, 8192, 3), xyz2 (4, 8192, 3) float32.
  d2[b, n, m] = ||xyz1[b,n] - xyz2[b,m]||^2
  out = mean_n(min_m d2) + mean_m(min_n d2)   (scalar, float32)

Sharding: 8 cores = (batch b in 0..3) x (half h of the N axis). Each core
computes, for its (b, h):
  - dist1[n] = min over all M of d2 for its 4096 rows (complete), and
  - partial dist2[m] = min over its 4096 rows (combined across the 2
    halves on the host with an elementwise min).

Device algorithm (per core):
  d2 is produced by a single K=16 bf16 matmul per tile using an exact-ish
  hi/lo decomposition (x = bf16(x) + bf16(x - bf16(x))):
    d2 = (-2 x1) . x2 + ||x1||^2 + ||x2||^2
  with features
    F1 = [y1h y1h y1l y1l n1h n1l 1 1]   (y1 = -2 x1, 16 rows)
    F2 = [x2h x2l x2h x2l 1 1 n2h n2l]
  so F1 . F2 accumulates all four cross products plus both norms in fp32
  PSUM. The same two feature buffers serve as stationary or moving for the
  two output orientations (A: m on partitions -> dist2; B: n on partitions
  -> dist1). Norms are computed on device (fp32 squares + ones-stationary
  matmul broadcast over all partitions).

  Reduction: 7 of every 8 [128, 2048] PSUM tiles are evacuated by ScalarE
  to SBUF as fp16 (cast on write) and folded by DVE tensor_tensor min at
  2 elems/cycle (2x_1p mode) plus a short reduce; the rest are min-reduced
  by DVE straight from PSUM at 1 elem/cycle. This splits the consumption
  of d2 across both engines (TENSOR_TENSOR_REDUCE and GpSimd
  tensor_tensor are unusable in this environment). fp16 only rounds the
  per-point minima (~2.4e-4 rel, unbiased); measured output error vs the
  fp32 reference is ~1e-5..2e-4.
"""

import numpy as np

import concourse.bass as bass
import concourse.tile as tile
from concourse import bacc, mybir
from concourse.bass_utils import run_bass_kernel_spmd

B, N, M = 4, 8192, 8192
NCORES = 8
NHALF = N // 2  # 4096 xyz1 rows per core

F32 = mybir.dt.float32
BF16 = mybir.dt.bfloat16
FP16 = mybir.dt.float16
BIG = 3.0e38

MT2 = M // 128  # 64 stationary chunks, orientation A
NT1 = NHALF // 128  # 32 stationary chunks, orientation B
C1 = NHALF // 512  # 8 moving chunks, orientation A
C2 = M // 512  # 16 moving chunks, orientation B

MIN = mybir.AluOpType.min
MULT = mybir.AluOpType.mult
SUB = mybir.AluOpType.subtract
BYP = mybir.AluOpType.bypass
AXIS_X = mybir.AxisListType.X


def _build_body(tc, x1t, x2t, dist1, dist2p, repeat, split, direct_mod):
    # Compute instructions may only start at partition 0/32/64/96 (BIR
    # verifier rule), so all feature rows are computed at partition base 0
    # and placed into their final partition rows via SBUF->SBUF DMAs
    # (DMAs are exempt from the rule).
    nc = tc.nc
    stt = nc.vector.scalar_tensor_tensor

    persist = tc.alloc_tile_pool(name="persist", bufs=1)
    prep = tc.alloc_tile_pool(name="prep", bufs=1)

    f1 = persist.tile([16, NHALF], BF16)
    f2 = persist.tile([16, M], BF16)
    acc_a = persist.tile([128, MT2 * 2], F32)
    acc_b = persist.tile([128, NT1 * 4], F32)
    d1out = persist.tile([128, NT1], F32)
    d2out = persist.tile([128, MT2], F32)

    ones_st = persist.tile([3, 128], F32)
    nc.vector.memset(ones_st[:], 1.0)
    nc.gpsimd.memset(f1[:], 1.0)
    nc.gpsimd.memset(f2[:], 1.0)

    def build_features(xin, feat, width, scale, hi_dup_rows, lo_rows, nrm_rows):
        # Shared-tag scratch so the x1 and x2 phases reuse the same SBUF.
        st = prep.tile([3, width], F32, tag="st", name="st")
        sq = prep.tile([3, width], F32, tag="sq", name="sq")
        lo = prep.tile([3, width], BF16, tag="lo", name="lo")
        nh = prep.tile([1, width], BF16, tag="nh", name="nh")
        nl = prep.tile([1, width], BF16, tag="nl", name="nl")

        nc.sync.dma_start(st[:], xin.ap())
        # ||x||^2: fp32 squares (ScalarE), then a ones-stationary matmul
        # broadcasts the per-point norm onto all 128 PSUM partitions; row 0
        # is split hi/lo straight out of PSUM.
        nc.scalar.activation(sq[:], st[:], mybir.ActivationFunctionType.Square)
        psn = tc.alloc_tile_pool(name="psn", bufs=2, space="PSUM")
        for c in range(width // 512):
            sl = slice(512 * c, 512 * (c + 1))
            pn = psn.tile([128, 512], F32, tag="pn", name="pn")
            nc.tensor.matmul(pn[:], ones_st[:], sq[:, sl], start=True, stop=True)
            nc.scalar.copy(nh[0:1, sl], pn[0:1, :])
            stt(nl[0:1, sl], pn[0:1, :], 0.0, nh[0:1, sl], BYP, SUB)
        psn.release()

        # hi/lo split of (scale * x) at partition base 0.
        nc.scalar.mul(feat[0:3, :], st[:], scale)  # hi -> rows 0-2
        stt(lo[:], st[:], scale, feat[0:3, :], MULT, SUB)

        # Place remaining rows (DMAs may start at any partition).
        nc.sync.dma_start(feat[hi_dup_rows[0] : hi_dup_rows[0] + 3, :], feat[0:3, :])
        for r in lo_rows:
            nc.sync.dma_start(feat[r : r + 3, :], lo[:])
        nc.sync.dma_start(feat[nrm_rows[0] : nrm_rows[0] + 1, :], nh[:])
        nc.sync.dma_start(feat[nrm_rows[1] : nrm_rows[1] + 1, :], nl[:])

    # F1 = [y1h y1h y1l y1l n1h n1l 1 1]   (y1 = -2 x1)
    build_features(x1t, f1, NHALF, -2.0, (3,), (6, 9), (12, 13))
    # F2 = [x2h x2l x2h x2l 1 1 n2h n2l]
    build_features(x2t, f2, M, 1.0, (6,), (3, 9), (14, 15))

    prep.release()

    def sweep(psum_pool, aux, stat, mov, n_stat, n_mov_groups, acc, split):
        # direct_mod: 1 of every direct_mod tiles stays on the direct DVE path.
        # One output orientation: for each 128-wide stationary chunk,
        # stream all moving chunks in groups of 4 x 512 into a 4-bank PSUM
        # tile, then min-reduce it over the free axis.
        # (TENSOR_TENSOR_REDUCE crashes the exec unit in this environment
        # and GpSimd tensor_tensor fails codegen, so the choices are DVE
        # tensor_reduce straight from PSUM, or - with `split` - an fp16
        # ScalarE evacuation + 2x-mode DVE folds for 7/8 of the tiles.)
        for s in range(n_stat):
            for g in range(n_mov_groups):
                ps = psum_pool.tile([128, 2048], F32, tag="ps", name="ps")
                for j in range(4):
                    c = g * 4 + j
                    nc.tensor.matmul(
                        ps[:, 512 * j : 512 * (j + 1)],
                        stat[:, 128 * s : 128 * (s + 1)],
                        mov[:, 512 * c : 512 * (c + 1)],
                        start=True,
                        stop=True,
                    )
                col = s * n_mov_groups + g
                if split and (col % direct_mod) != 0:
                    # Off-critical-path consumption: ScalarE evacuates the
                    # tile to SBUF as fp16 (cast on write); DVE then folds
                    # at 2 elems/cycle (2x_1p mode, 16-bit dtypes only)
                    # and finishes with a short 512-wide reduce. fp16 only
                    # rounds the min values (~2.4e-4 rel, unbiased).
                    gt = aux.tile([128, 2048], FP16, tag="gt", name="gt", bufs=3)
                    nc.scalar.copy(gt[:], ps[:])
                    gf = aux.tile([128, 1024], FP16, tag="gf", name="gf", bufs=2)
                    nc.vector.tensor_tensor(
                        gf[:], gt[:, 0:1024], gt[:, 1024:2048], op=MIN
                    )
                    gf2 = aux.tile([128, 512], FP16, tag="gf2", name="gf2", bufs=2)
                    nc.vector.tensor_tensor(
                        gf2[:], gf[:, 0:512], gf[:, 512:1024], op=MIN
                    )
                    nc.vector.tensor_reduce(
                        acc[:, col : col + 1], gf2[:], axis=AXIS_X, op=MIN
                    )
                else:
                    nc.vector.tensor_reduce(
                        acc[:, col : col + 1], ps[:], axis=AXIS_X, op=MIN
                    )

    aux = tc.alloc_tile_pool(name="aux", bufs=1)
    ps_pool = tc.alloc_tile_pool(name="ps_pool", bufs=2, space="PSUM")

    def one_pass():
        sweep(ps_pool, aux, f2, f1, MT2, C1 // 4, acc_a, split)  # A: m on parts
        sweep(ps_pool, aux, f1, f2, NT1, C2 // 4, acc_b, split)  # B: n on parts

    if repeat == 1:
        one_pass()
    else:
        # Benchmarking mode: re-run the main loop on-device so its cost
        # dominates the fixed host/RPC dispatch overhead.
        with tc.For_i(0, repeat, 1):
            one_pass()

    ps_pool.release()
    aux.release()

    nc.vector.tensor_reduce(
        d2out[:], acc_a[:].rearrange("p (a b) -> p a b", b=2), axis=AXIS_X, op=MIN
    )
    nc.vector.tensor_reduce(
        d1out[:], acc_b[:].rearrange("p (a b) -> p a b", b=4), axis=AXIS_X, op=MIN
    )
    nc.vector.tensor_scalar_max(d2out[:], d2out[:], 0.0)
    nc.vector.tensor_scalar_max(d1out[:], d1out[:], 0.0)
    nc.sync.dma_start(dist1.ap(), d1out[:])
    nc.sync.dma_start(dist2p.ap(), d2out[:])

    persist.release()


def build_nc(repeat=1, split=True, direct_mod=8):
    nc = bacc.Bacc(
        "TRN2", target_bir_lowering=False, debug=False, num_devices=NCORES
    )
    x1t = nc.dram_tensor("x1t", [3, NHALF], F32, kind="ExternalInput")
    x2t = nc.dram_tensor("x2t", [3, M], F32, kind="ExternalInput")
    dist1 = nc.dram_tensor("dist1", [128, NT1], F32, kind="ExternalOutput")
    dist2p = nc.dram_tensor("dist2p", [128, MT2], F32, kind="ExternalOutput")
    with tile.TileContext(nc) as tc:
        _build_body(tc, x1t, x2t, dist1, dist2p, repeat, split, direct_mod)
    nc.compile()
    return nc


_NC_CACHE = {}


def get_nc(repeat=1, split=True, direct_mod=8):
    key = (repeat, split, direct_mod)
    if key not in _NC_CACHE:
        _NC_CACHE[key] = build_nc(repeat, split, direct_mod)
    return _NC_CACHE[key]


def make_in_maps(xyz1, xyz2):
    in_maps = []
    for c in range(NCORES):
        b, h = divmod(c, 2)
        x1 = xyz1[b, h * NHALF : (h + 1) * NHALF, :]
        in_maps.append(
            {
                "x1t": np.ascontiguousarray(x1.T),
                "x2t": np.ascontiguousarray(xyz2[b].T),
            }
        )
    return in_maps


def combine(results):
    s1 = 0.0
    s2 = 0.0
    for b in range(B):
        r0, r1 = results[2 * b], results[2 * b + 1]
        s1 += r0["dist1"].T.reshape(-1).sum(dtype=np.float64)
        s1 += r1["dist1"].T.reshape(-1).sum(dtype=np.float64)
        d2 = np.minimum(r0["dist2p"].T.reshape(-1), r1["dist2p"].T.reshape(-1))
        s2 += d2.sum(dtype=np.float64)
    return np.float32(s1 / (B * N) + s2 / (B * M))


def kernel(xyz1, xyz2):
    xyz1 = np.asarray(xyz1, dtype=np.float32)
    xyz2 = np.asarray(xyz2, dtype=np.float32)
    nc = get_nc()
    res = run_bass_kernel_spmd(nc, make_in_maps(xyz1, xyz2), core_ids=list(range(NCORES)))
    return combine(res.results)


if __name__ == "__main__":
    rng = np.random.default_rng(0)
    a = rng.standard_normal((B, N, 3), dtype=np.float32)
    b = rng.standard_normal((B, M, 3), dtype=np.float32)
    print("kernel:", kernel(a, b))

